# revision 4
# baseline (speedup 1.0000x reference)
"""Trainium2 Bass kernel for nn_CustomLossMinMax.

Computes, over full inputs pos_outputs [N,L], neg_outputs [M,L], p [N,L]
(N=M=8192, L=2048, f32):

    wpos[i]   = sum_l pos[i,l] * p[i,l]
    negmax[j] = max_l neg[j,l]
    out       = sum_ij relu(1 - wpos[i] + negmax[j]) / (N*M)

Sharding (8 cores): rows of pos/p and rows of neg are split 1024/core.
Each core computes its wpos shard and negmax shard, AllGathers the tiny
negmax vector (32 KiB total), then accumulates the pairwise hinge for its
own 1024 i-rows against all 8192 j's. Per-core partial sums [128, 8] are
summed on the host (the scalar all-reduce step) and scaled by 1/(N*M).

Engine split per core: DMA streams 24 MiB (the roofline); DVE does the
fused mul+reduce (wpos), row-max (negmax), and 2 of 8 pairwise tiles via
tensor_scalar(add,max)+accum; ACT does 6 of 8 pairwise tiles via
Relu-activation with per-partition bias and free-dim accumulation; GPSIMD
broadcasts the gathered negmax row to all partitions.

All i/j orderings inside the kernel are permutations of the reference
ordering; the final scalar sum is permutation-invariant.
"""
import sys
import numpy as np

for _p in ("/opt/trn_rl_repo", "/root/.axon_site/_ro/trn_rl_repo"):
    if _p not in sys.path:
        sys.path.insert(0, _p)

from concourse import bacc, mybir, tile  # noqa: E402
from concourse import bass_utils  # noqa: E402

N_CORES = 8
N, M, L = 8192, 8192, 2048
ROWS = N // N_CORES          # 1024 rows per core for pos/p and neg
T = ROWS // 128              # 8 row-tiles of 128 partitions per core
F32 = mybir.dt.float32

# pairwise row-tiles handled by the scalar (ACT) engine; the rest go to DVE
ACT_TILES = 6

_cache = {}


def _build():
    nc = bacc.Bacc("TRN2", target_bir_lowering=False, debug=False,
                   enable_asserts=True, num_devices=N_CORES)
    pos = nc.dram_tensor("pos", [ROWS, L], F32, kind="ExternalInput").ap()
    p = nc.dram_tensor("p", [ROWS, L], F32, kind="ExternalInput").ap()
    neg = nc.dram_tensor("neg", [ROWS, L], F32, kind="ExternalInput").ap()
    out = nc.dram_tensor("partial", [128, T], F32, kind="ExternalOutput").ap()

    pos_t = pos.rearrange("(t p) l -> t p l", p=128)
    p_t = p.rearrange("(t p) l -> t p l", p=128)
    neg_t = neg.rearrange("(t p) l -> t p l", p=128)

    with tile.TileContext(nc) as tc:
        with tc.tile_pool(name="io", bufs=2) as io_pool, \
             tc.tile_pool(name="big", bufs=1) as big_pool, \
             tc.tile_pool(name="small", bufs=1) as small_pool, \
             tc.tile_pool(name="dram", bufs=1, space="DRAM") as dpool:

            # ---- Phase 1: per-core negmax shard ------------------------
            negmax_sb = small_pool.tile([128, T], F32)
            for t in range(T):
                ntile = io_pool.tile([128, L], F32, tag="neg")
                nc.sync.dma_start(ntile[:], neg_t[t])
                nc.vector.tensor_reduce(negmax_sb[:, t:t + 1], ntile[:],
                                        axis=mybir.AxisListType.X,
                                        op=mybir.AluOpType.max)

            # fold the hinge's "+1" into the gathered vector: 1 + negmax
            negmax1_sb = small_pool.tile([128, T], F32)
            nc.vector.tensor_scalar_add(negmax1_sb[:], negmax_sb[:], 1.0)

            # ---- Phase 2: AllGather (1 + negmax) across the 8 cores ----
            cc_in = dpool.tile([128, T], F32)
            cc_out = dpool.tile([128 * N_CORES, T], F32)
            nc.sync.dma_start(cc_in[:], negmax1_sb[:])
            nc.gpsimd.collective_compute(
                "AllGather",
                mybir.AluOpType.bypass,
                ins=[cc_in[:].opt()],
                outs=[cc_out[:].opt()],
                replica_groups=[list(range(N_CORES))],
            )
            # full (permuted) negmax vector -> one SBUF row -> all partitions
            row = big_pool.tile([1, M], F32, tag="row")
            nc.sync.dma_start(row[:], cc_out[:].rearrange("a b -> (a b)")
                              .rearrange("(a b) -> a b", a=1))
            bcast = big_pool.tile([128, M], F32, tag="bcast")
            nc.gpsimd.partition_broadcast(bcast[:], row[:])

            # ---- Phase 3: a = -wpos per row-tile (fused on DVE) --------
            # scalar_tensor_tensor: out = (pos * -1) * p, accum = sum(out)
            a_sb = small_pool.tile([128, T], F32)
            for t in range(T):
                ptile = io_pool.tile([128, L], F32, tag="pos")
                wtile = io_pool.tile([128, L], F32, tag="p")
                nc.sync.dma_start(ptile[:], pos_t[t])
                nc.sync.dma_start(wtile[:], p_t[t])
                scr = io_pool.tile([128, L], F32, tag="ttr_scr")
                nc.vector.scalar_tensor_tensor(
                    out=scr[:], in0=ptile[:], scalar=-1.0, in1=wtile[:],
                    op0=mybir.AluOpType.mult, op1=mybir.AluOpType.mult,
                    accum_out=a_sb[:, t:t + 1])

            # ---- Phase 4: pairwise hinge, split across ACT and DVE -----
            # ACT: out = Relu(bcast + a) with free-dim accumulate.
            # DVE: out = (bcast + a) max 0-broadcast, with accumulate
            #      (scalar_tensor_tensor; op1 is elementwise, accum is add).
            acc = small_pool.tile([128, T], F32)
            act_scr = big_pool.tile([128, M], F32, tag="act_scr")
            dve_scr = big_pool.tile([128, M], F32, tag="dve_scr")
            zeros = small_pool.tile([128, 1], F32)
            nc.vector.memset(zeros[:], 0.0)
            zeros_b = zeros[:].broadcast_to((128, M))
            for t in range(T):
                if t < ACT_TILES:
                    nc.scalar.activation(
                        act_scr[:], bcast[:],
                        mybir.ActivationFunctionType.Relu,
                        bias=a_sb[:, t:t + 1], scale=1.0,
                        accum_out=acc[:, t:t + 1])
                else:
                    nc.vector.scalar_tensor_tensor(
                        out=dve_scr[:], in0=bcast[:],
                        scalar=a_sb[:, t:t + 1], in1=zeros_b,
                        op0=mybir.AluOpType.add, op1=mybir.AluOpType.max,
                        accum_out=acc[:, t:t + 1])

            nc.sync.dma_start(out, acc[:])
    nc.compile()
    return nc


def kernel(pos_outputs: np.ndarray, neg_outputs: np.ndarray,
           p: np.ndarray) -> np.ndarray:
    if "nc" not in _cache:
        _cache["nc"] = _build()
    nc = _cache["nc"]

    pos_outputs = np.ascontiguousarray(pos_outputs, dtype=np.float32)
    neg_outputs = np.ascontiguousarray(neg_outputs, dtype=np.float32)
    p = np.ascontiguousarray(p, dtype=np.float32)

    in_maps = []
    for c in range(N_CORES):
        sl = slice(c * ROWS, (c + 1) * ROWS)
        in_maps.append({
            "pos": pos_outputs[sl],
            "p": p[sl],
            "neg": neg_outputs[sl],
        })
    res = bass_utils.run_bass_kernel_spmd(nc, in_maps,
                                          core_ids=list(range(N_CORES)))
    total = 0.0
    for c in range(N_CORES):
        total += res.results[c]["partial"].astype(np.float64).sum()
    return np.asarray(total / (float(N) * float(M)), dtype=np.float32)
